# revision 23
# baseline (speedup 1.0000x reference)
"""Multi-head causal attention (QKV proj + RoPE + softmax attention + out proj)
as a distributed Bass kernel on 8 Trainium2 NeuronCores.

Sharding: tensor-parallel over heads. Each core owns 2 of the 16 heads:
it computes Q/K/V for its heads from the full (replicated) input, runs
attention (token-major PV with a ones-column for the softmax denominator),
and the normalized per-head attention outputs are AllGather'd token-major.
Each core then computes a 256-column slice of the final output projection
with the AllGather'd activations loaded via DMA-transpose (X-bar), so no
PE transposes are needed anywhere. The out-projection runs "flipped"
(weights stationary, tokens moving, N=512) and emits out^T; the host
transposes and concatenates the 8 column slices.

All matmuls run in bf16 (fp32 PSUM accumulation); softmax runs without
max-subtraction (scores are ~N(0,1) here, so exp is safe in fp32).
"""

import math
import numpy as np
import ml_dtypes

B, S, D, H = 2, 2048, 2048, 16
HD = 128                  # head dim
P = 128                   # SBUF partitions
NT = B * S                # 4096 tokens
N_CORES = 8
HPC = H // N_CORES        # heads per core
DQ = HPC * HD             # 256 q/k/v rows per core
KC = D // P               # 16 contraction chunks
TCH = 512                 # token chunk in QKV projection
NTC = NT // TCH           # 8
SBK = S // P              # 16 key blocks per batch
QCH = 512                 # q chunk in attention
SH = S // 2               # AllGather half (1024 tokens)
BF = ml_dtypes.bfloat16

_cache = {}


def _vaug_col(b, i, h):
    # column base of V chunk (batch b, s-chunk i, head h) in the vaug tile
    return ((b * SBK + i) * HPC + h) * (HD + 1)


def _attn_col(b, i, h):
    # column base of the normalized attention-out chunk (token-major)
    return ((b * SBK + i) * HPC + h) * HD


def _build(mask_mode):
    from concourse import bacc
    import concourse.mybir as mybir
    import concourse.tile as tile
    from concourse.tile_rust import add_dep_helper

    bf = mybir.dt.bfloat16
    f32 = mybir.dt.float32
    EXP = mybir.ActivationFunctionType.Exp
    scale = 1.0 / math.sqrt(HD)
    causal = mask_mode == "causal"

    nc = bacc.Bacc("TRN2", target_bir_lowering=False, debug=False,
                   num_devices=N_CORES)

    xRe = nc.declare_dram_parameter("xRe", [P, NTC * KC * TCH], bf,
                                    isOutput=False)
    wqp = nc.declare_dram_parameter("wqp", [P, KC * DQ], bf, isOutput=False)
    wkp = nc.declare_dram_parameter("wkp", [P, KC * DQ], bf, isOutput=False)
    wvp = nc.declare_dram_parameter("wvp", [P, KC * DQ], bf, isOutput=False)
    wop = nc.declare_dram_parameter("wop", [P, KC * DQ], bf, isOutput=False)
    cro = nc.declare_dram_parameter("cro", [P, NT], bf, isOutput=False)
    sro = nc.declare_dram_parameter("sro", [P, NT], bf, isOutput=False)
    cst = nc.declare_dram_parameter("cst", [P, 2 * P], bf, isOutput=False)
    mskT = None
    if mask_mode == "general":
        mskT = nc.declare_dram_parameter("mskT", [S, S], bf, isOutput=False)
    outT = nc.declare_dram_parameter("outT", [DQ, NT], f32, isOutput=True)
    import os as _os
    _dbg = _os.environ.get("KDBG", "0") == "1"
    attnD = agD = None
    if _dbg:
        attnD = nc.declare_dram_parameter("attnD", [P, B * SBK * HPC * HD],
                                          bf, isOutput=True)
        agD = nc.declare_dram_parameter("agD", [N_CORES * SH, DQ], bf,
                                        isOutput=True)

    rg = [list(range(N_CORES))]

    with tile.TileContext(nc) as tc:
        with (
            tc.tile_pool(name="per", bufs=1) as per,
            tc.tile_pool(name="stage", bufs=8) as stage,
            tc.tile_pool(name="dram", bufs=1, space="DRAM") as drp,
            tc.tile_pool(name="ptp",
                         bufs=(1 if mask_mode == "general" else 2)) as ptp,
            tc.tile_pool(name="mkp", bufs=4) as mkp,
        ):
            # ---------------- persistent SBUF ----------------
            q_sb = per.tile([P, HPC * NT], bf)       # d-major Q, head h at h*NT
            k_sb = per.tile([P, HPC * NT], bf)
            vaug_sb = per.tile([P, B * SBK * HPC * (HD + 1)], bf)
            attn_sb = per.tile([P, B * SBK * HPC * HD], bf)  # token-major out
            wo_sb = per.tile([P, KC * DQ], bf)
            cst_sb = per.tile([P, 2 * P], bf)
            perm = cst_sb[:, 0:P]
            tri01 = cst_sb[:, P:2 * P]

            nc.sync.dma_start(out=cst_sb[:], in_=cst[:, :])
            # ones columns for the PV denominator trick
            nc.gpsimd.memset(vaug_sb[:], 1.0)

            # phase-A-scoped SBUF
            wq_sb, free_wq = tc.tile([P, KC * DQ], bf, name="wq_sb")
            wk_sb, free_wk = tc.tile([P, KC * DQ], bf, name="wk_sb")
            wv_sb, free_wv = tc.tile([P, KC * DQ], bf, name="wv_sb")
            cro_sb, free_cro = tc.tile([P, NT], bf, name="cro_sb")
            sro_sb, free_sro = tc.tile([P, NT], bf, name="sro_sb")

            # weights + rope tables stream in on the scalar (ACT) HWDGE queue
            nc.scalar.dma_start(out=wq_sb[:], in_=wqp[:, :])
            nc.scalar.dma_start(out=wk_sb[:], in_=wkp[:, :])
            nc.scalar.dma_start(out=wv_sb[:], in_=wvp[:, :])
            nc.scalar.dma_start(out=cro_sb[:], in_=cro[:, :])
            nc.scalar.dma_start(out=sro_sb[:], in_=sro[:, :])

            # DRAM bounce buffers for the AllGather: one per (batch, s-half)
            # token-major: [SH tokens, DQ head-dims]
            bounce = [[drp.tile([SH, DQ], bf, name=f"bounce{b}{f}",
                                tag=f"bounce{b}{f}")
                       for f in range(2)] for b in range(B)]
            ag = [[drp.tile([N_CORES * SH, DQ], bf, addr_space="Shared",
                            name=f"ag{b}{f}", tag=f"ag{b}{f}")
                   for f in range(2)] for b in range(B)]

            def attention(b, half, p_st, st_tag, p_pv, pv_tag):
                for h in range(HPC):
                    qoff = h * NT + b * S
                    for qc in (half * 2, half * 2 + 1):
                        n_s = SBK if not causal else 4 * qc + 4
                        pt = ptp.tile([P, SBK * QCH], bf, tag="pt",
                                      name=f"pt{b}{h}{qc}")
                        for sb in range(n_s):
                            stp = p_st.tile([P, QCH], f32, tag=st_tag,
                                            name=f"st{b}{h}{qc}{sb}")
                            nc.tensor.matmul(
                                stp[:],
                                k_sb[:, qoff + sb * P:qoff + (sb + 1) * P],
                                q_sb[:, qoff + qc * QCH:qoff + (qc + 1) * QCH])
                            if mask_mode == "general":
                                mk = mkp.tile([P, QCH], bf, tag="mk",
                                              name=f"mk{b}{h}{qc}{sb}")
                                nc.sync.dma_start(
                                    out=mk[:],
                                    in_=mskT[sb * P:(sb + 1) * P,
                                             qc * QCH:(qc + 1) * QCH])
                                nc.vector.tensor_add(stp[:], stp[:], mk[:])
                            # causal: skip exp on fully-masked q columns
                            off = (sb - 4 * qc) * P \
                                if (causal and sb > 4 * qc) else 0
                            if _os.environ.get("KFULLEXP", "0") == "1":
                                off = 0
                            nc.scalar.activation(
                                pt[:, sb * QCH + off:(sb + 1) * QCH],
                                stp[:, off:QCH], EXP, scale=scale)
                        if causal:
                            for j in range(QCH // P):
                                sb = 4 * qc + j
                                c0 = sb * QCH + j * P
                                nc.vector.tensor_mul(
                                    pt[:, c0:c0 + P], pt[:, c0:c0 + P], tri01)
                        for jj in range(QCH // P):
                            qb = 4 * qc + jj
                            n_pv = SBK if not causal else qb + 1
                            pv = p_pv.tile([P, HD + 1], f32, tag=pv_tag,
                                           name=f"pv{b}{h}{qb}")
                            for sb in range(n_pv):
                                nc.tensor.matmul(
                                    pv[:],
                                    pt[:, sb * QCH + jj * P:
                                       sb * QCH + (jj + 1) * P],
                                    vaug_sb[:, _vaug_col(b, sb, h):
                                            _vaug_col(b, sb, h) + HD + 1],
                                    start=(sb == 0), stop=(sb == n_pv - 1))
                            rec = stage.tile([P, 1], f32, tag="rec",
                                             name=f"rec{b}{h}{qb}")
                            nc.vector.reciprocal(rec[:], pv[:, HD:HD + 1])
                            # normalize straight into token-major attn_sb
                            nc.vector.tensor_scalar_mul(
                                attn_sb[:, _attn_col(b, qb, h):
                                        _attn_col(b, qb, h) + HD],
                                pv[:, 0:HD], rec[:])
                # bounce out token-major: row block i_local <- partition dim
                for il in range(SH // P):
                    i = half * (SH // P) + il
                    nc.gpsimd.dma_start(
                        out=bounce[b][half][il * P:(il + 1) * P, :],
                        in_=attn_sb[:, _attn_col(b, i, 0):
                                    _attn_col(b, i, 0) + HPC * HD])
                nc.gpsimd.collective_compute(
                    "AllGather", mybir.AluOpType.bypass,
                    replica_groups=rg,
                    ins=[bounce[b][half].opt()], outs=[ag[b][half].opt()])

            def outproj(b, half, p_op, ags, ostp):
                # load the AllGather'd activations d-major via DMA-transpose
                agts = []
                for kk in range(KC):
                    r, h = kk // HPC, kk % HPC
                    agt = ags.tile([P, SH], bf, tag="agt",
                                   name=f"agt{b}{half}{kk}")
                    # NOTE: X-bar transpose DMAs must all go through ONE
                    # HWDGE queue — concurrent transposes from sync+scalar
                    # queues corrupt data (verified on HW).
                    eng = nc.sync
                    eng.dma_start(
                        out=agt[:],
                        in_=ag[b][half][r * SH:(r + 1) * SH,
                                        h * HD:(h + 1) * HD],
                        transpose=True)
                    agts.append(agt)
                for tg in range(SH // QCH):
                    ops = [p_op.tile([P, QCH], f32, tag="op",
                                     name=f"op{b}{half}{tg}{oc}")
                           for oc in range(2)]
                    for kk in range(KC):
                        for oc in range(2):
                            nc.tensor.matmul(
                                ops[oc],
                                wo_sb[:, kk * DQ + oc * P:
                                      kk * DQ + (oc + 1) * P],
                                agts[kk][:, tg * QCH:(tg + 1) * QCH],
                                start=(kk == 0), stop=(kk == KC - 1))
                    t0 = b * S + half * SH + tg * QCH
                    for oc in range(2):
                        ost = ostp.tile([P, QCH], f32, tag="ost",
                                        name=f"ost{b}{half}{tg}{oc}")
                        nc.vector.tensor_copy(ost[:], ops[oc][:])
                        nc.gpsimd.dma_start(
                            out=outT[oc * P:(oc + 1) * P, t0:t0 + QCH],
                            in_=ost[:])

            # ---------------- phase A: QKV projection + RoPE ----------------
            # weave points for attention (needs K/V of its key range done)
            import os as _os
            _wv = _os.environ.get("KWEAVE", "2467")
            if causal:
                _pts = {"2": (0, 0), "4": (0, 1), "6": (1, 0), "7": (1, 1)}
                weave = {int(c): _pts[c] for c in "2467" if c in _wv}
            else:
                weave = {3: (0, 0), 5: (0, 1), 7: (1, 0)}
            with (
                tc.tile_pool(name="ps_qk", bufs=4, space="PSUM") as ps_qk,
                tc.tile_pool(name="ps_v", bufs=2, space="PSUM") as ps_v,
                tc.tile_pool(name="ps_ax", bufs=2, space="PSUM") as ps_ax,
                tc.tile_pool(name="xs", bufs=3) as xs,
                tc.tile_pool(name="rt", bufs=4) as rt,
            ):
                for tci in range(NTC):
                    t0 = tci * TCH
                    x_t = []
                    for xh in range(2):
                        xt_h = xs.tile([P, KC * TCH // 2], bf, tag="xt",
                                       name=f"xt{tci}_{xh}")
                        nc.sync.dma_start(
                            out=xt_h[:],
                            in_=xRe[:, (tci * KC + xh * KC // 2) * TCH:
                                    (tci * KC + (xh + 1) * KC // 2) * TCH])
                        x_t.append(xt_h)
                    qp = [ps_qk.tile([P, TCH], f32, tag="qkps",
                                     name=f"qp{tci}_{m}") for m in range(HPC)]
                    kp = [ps_qk.tile([P, TCH], f32, tag="qkps",
                                     name=f"kp{tci}_{m}") for m in range(HPC)]
                    vp = [ps_v.tile([P, 2 * DQ], f32, tag="vps",
                                    name=f"vp{tci}_{u}") for u in range(2)]
                    vfirst = {}
                    for kk in range(KC):
                        xth = x_t[kk // (KC // 2)]
                        kkl = kk % (KC // 2)
                        xt = xth[:, kkl * TCH:(kkl + 1) * TCH]
                        st = (kk == 0)
                        sp = (kk == KC - 1)
                        for m in range(HPC):
                            nc.tensor.matmul(
                                qp[m],
                                wq_sb[:, kk * DQ + m * HD:
                                      kk * DQ + (m + 1) * HD],
                                xt, start=st, stop=sp)
                            nc.tensor.matmul(
                                kp[m],
                                wk_sb[:, kk * DQ + m * HD:
                                      kk * DQ + (m + 1) * HD],
                                xt, start=st, stop=sp)
                        for tb in range(TCH // P):
                            mm = nc.tensor.matmul(
                                vp[tb // 2][:, (tb % 2) * DQ:(tb % 2 + 1) * DQ],
                                xth[:, kkl * TCH + tb * P:
                                    kkl * TCH + (tb + 1) * P],
                                wv_sb[:, kk * DQ:(kk + 1) * DQ],
                                start=(st and tb % 2 == 0), stop=sp,
                                skip_group_check=(tb % 2 == 1))
                            if kk == 0:
                                vfirst[tb] = mm
                    for u in range(2):
                        # the second group's first MM must follow the bank
                        # clear done by the first group's start=True MM
                        add_dep_helper(vfirst[u * 2 + 1].ins,
                                       vfirst[u * 2].ins, sync=False,
                                       reason="bank-clear before 2nd V group")
                    # V: copy token-major psum into vaug (per head), bf16 (DVE)
                    for tb in range(TCH // P):
                        tglob = t0 + tb * P
                        b = tglob // S
                        i = (tglob % S) // P
                        for h in range(HPC):
                            c0 = _vaug_col(b, i, h)
                            nc.vector.tensor_copy(
                                vaug_sb[:, c0:c0 + HD],
                                vp[tb // 2][:, (tb % 2) * DQ + h * HD:
                                            (tb % 2) * DQ + (h + 1) * HD])
                    # RoPE on Q and K (d-major): out = C*z + Sro*pairswap(z)
                    for (ps_list, dst) in ((qp, q_sb), (kp, k_sb)):
                        for m in range(HPC):
                            zb = stage.tile([P, TCH], bf, tag="zb",
                                            name=f"zb{tci}{m}")
                            nc.vector.tensor_copy(zb[:], ps_list[m][:])
                            zs = ps_qk.tile([P, TCH], f32, tag="qkps",
                                            name=f"zs{tci}{m}")
                            nc.tensor.matmul(zs[:], perm, zb[:])
                            t1 = rt.tile([P, TCH], f32, tag="t1",
                                         name=f"t1{tci}{m}")
                            t2 = rt.tile([P, TCH], f32, tag="t2",
                                         name=f"t2{tci}{m}")
                            nc.vector.tensor_mul(t1[:], zb[:],
                                                 cro_sb[:, t0:t0 + TCH])
                            nc.vector.tensor_mul(t2[:], zs[:],
                                                 sro_sb[:, t0:t0 + TCH])
                            nc.vector.tensor_add(
                                dst[:, m * NT + t0:m * NT + t0 + TCH],
                                t1[:], t2[:])

                    if tci in weave:
                        wb, wh = weave[tci]
                        attention(wb, wh, ps_ax, "ax", ps_ax, "ax")

            free_sro(); free_cro(); free_wv(); free_wk(); free_wq()

            # out-proj weights: needed only after the first AllGather
            nc.sync.dma_start(out=wo_sb[:], in_=wop[:, :])

            # ---------------- phase B: remaining attention + out-proj ------
            with (
                tc.tile_pool(name="ps_st", bufs=3, space="PSUM") as ps_st,
                tc.tile_pool(name="ps_pv", bufs=2, space="PSUM") as ps_pv,
                tc.tile_pool(name="ps_op", bufs=3, space="PSUM") as ps_op,
                tc.tile_pool(name="ags", bufs=32) as ags,
                tc.tile_pool(name="ostp", bufs=6) as ostp,
            ):
                woven = set(weave.values())
                _order = ((0, 0), (0, 1), (1, 0), (1, 1))
                if _os.environ.get("KREV", "0") == "1":
                    _order = ((0, 1), (1, 1), (0, 0), (1, 0))
                for bh in _order:
                    if bh not in woven:
                        attention(bh[0], bh[1], ps_st, "st", ps_pv, "pv")
                outproj(0, 0, ps_op, ags, ostp)
                outproj(0, 1, ps_op, ags, ostp)
                outproj(1, 0, ps_op, ags, ostp)
                outproj(1, 1, ps_op, ags, ostp)
                if _dbg:
                    nc.sync.dma_start(out=attnD[:, :], in_=attn_sb[:])
                    nc.gpsimd.dma_start(out=agD[:, :], in_=ag[0][1][:, :])

    nc.compile()
    return nc


def _host_prep(inputs):
    x = np.ascontiguousarray(np.asarray(inputs["x"], np.float32).reshape(NT, D))
    wq = np.asarray(inputs["wq"], np.float32)
    wk = np.asarray(inputs["wk"], np.float32)
    wv = np.asarray(inputs["wv"], np.float32)
    wo = np.asarray(inputs["wo"], np.float32)
    cos = np.asarray(inputs["freqs_cos"], np.float32)
    sin = np.asarray(inputs["freqs_sin"], np.float32)
    mask = np.asarray(inputs["mask"], np.float32).reshape(S, S)

    tril = np.tril(np.ones((S, S), bool))
    if not mask.any():
        mode = "zeros"
    elif (mask[tril] == 0).all() and (mask[~tril] <= -1e8).all():
        mode = "causal"
    else:
        mode = "general"

    # x packed tci-major: xRe[p, (tci*KC+kk)*TCH + t] = x[tci*TCH+t, kk*P+p]
    xRe = np.ascontiguousarray(
        x.reshape(NTC, TCH, KC, P).transpose(3, 0, 2, 1)
        .reshape(P, NTC * KC * TCH).astype(BF))

    C = np.empty((P, S), np.float32)
    Sn = np.empty((P, S), np.float32)
    C[0::2] = cos.T
    C[1::2] = cos.T
    Sn[0::2] = -sin.T
    Sn[1::2] = sin.T
    cro = np.ascontiguousarray(np.concatenate([C, C], axis=1).astype(BF))
    sro = np.ascontiguousarray(np.concatenate([Sn, Sn], axis=1).astype(BF))

    cst = np.zeros((P, 2 * P), np.float32)
    pr = np.zeros((P, P), np.float32)
    idx = np.arange(0, P, 2)
    pr[idx, idx + 1] = 1.0
    pr[idx + 1, idx] = 1.0
    cst[:, 0:P] = pr
    cst[:, P:2 * P] = np.triu(np.ones((P, P), np.float32))
    cst = np.ascontiguousarray(cst.astype(BF))

    def wpack(w, r):
        # [P, KC*DQ] with block kk at cols kk*DQ: wp[p, kk*DQ+c] = w[r,:].T[kk*P+p, c]
        wT = np.ascontiguousarray(w[r, :].T)
        return np.ascontiguousarray(
            wT.reshape(KC, P, DQ).transpose(1, 0, 2)
            .reshape(P, KC * DQ).astype(BF))

    in_maps = []
    for c in range(N_CORES):
        r = slice(c * DQ, (c + 1) * DQ)
        m = {
            "xRe": xRe,
            "wqp": wpack(wq, r),
            "wkp": wpack(wk, r),
            "wvp": wpack(wv, r),
            "wop": wpack(wo, r),
            "cro": cro,
            "sro": sro,
            "cst": cst,
        }
        if mode == "general":
            m["mskT"] = np.ascontiguousarray(
                (mask.T * math.sqrt(HD)).astype(BF))
        in_maps.append(m)
    return mode, in_maps


LAST_RESULT = None


def kernel(**inputs):
    global LAST_RESULT
    from concourse.bass_utils import run_bass_kernel_spmd

    mode, in_maps = _host_prep(inputs)
    if mode not in _cache:
        _cache[mode] = _build(mode)
    nc = _cache[mode]

    res = run_bass_kernel_spmd(nc, in_maps, list(range(N_CORES)))
    LAST_RESULT = res

    out_full = np.empty((NT, D), np.float32)
    for c in range(N_CORES):
        out_full[:, c * DQ:(c + 1) * DQ] = res.results[c]["outT"].T
    return out_full.reshape(B, S, D)


# revision 33
# speedup vs baseline: 1.0760x; 1.0760x over previous
"""Multi-head causal attention (QKV proj + RoPE + softmax attention + out proj)
as a distributed Bass kernel on 8 Trainium2 NeuronCores.

Sharding: tensor-parallel over heads (2 of 16 heads per core).

Pipeline (v3): one token chunk (512) per step — QKV projection + RoPE for
the chunk, then immediately the attention q-chunk it enables (causal), so
the 4 AllGathers fire evenly through the kernel instead of bunching at the
end. Everything lives in ONE tile-pool scope (no mid-kernel pool barrier).

Layouts: Q/K d-major in SBUF; V token-major with a ones column (softmax
denominator comes free out of the PV matmul); attention output token-major,
normalized in a single DVE op; AllGather is token-major h-major rows so the
out-projection can read it back d-major via contiguous X-bar DMA-transposes
(all on ONE HWDGE queue — concurrent transposes from two queues corrupt).
Out-projection runs flipped (wo stationary, tokens moving, N=512) emitting
out^T; the host transposes and concatenates the 8 column slices.

All matmuls bf16 (fp32 PSUM); softmax without max-subtraction (scores ~N(0,1)).
"""

import math
import os as _os
import numpy as np
import ml_dtypes

B, S, D, H = 2, 2048, 2048, 16
HD = 128                  # head dim
P = 128                   # SBUF partitions
NT = B * S                # 4096 tokens
N_CORES = 8
HPC = H // N_CORES        # heads per core
DQ = HPC * HD             # 256 q/k/v rows per core
KC = D // P               # 16 contraction chunks
TCH = 512                 # token chunk
NTC = NT // TCH           # 8
SBK = S // P              # 16 key blocks per batch
QCH = 512                 # q chunk in attention
SH = S // 2               # AllGather half (1024 tokens)
BF = ml_dtypes.bfloat16

_cache = {}


def _vaug_col(b, i, h):
    return ((b * SBK + i) * HPC + h) * (HD + 1)


def _attn_col(b, i, h):
    return ((b * SBK + i) * HPC + h) * HD


def _build(mask_mode):
    from concourse import bacc
    import concourse.mybir as mybir
    import concourse.tile as tile
    from concourse.tile_rust import add_dep_helper

    bf = mybir.dt.bfloat16
    f32 = mybir.dt.float32
    EXP = mybir.ActivationFunctionType.Exp
    scale = 1.0 / math.sqrt(HD)
    causal = mask_mode == "causal"
    fullexp = _os.environ.get("KFULLEXP", "0") == "1"
    dbg = _os.environ.get("KDBG", "0") == "1"

    nc = bacc.Bacc("TRN2", target_bir_lowering=False, debug=False,
                   num_devices=N_CORES)

    xRe = nc.declare_dram_parameter("xRe", [P, NTC * KC * TCH], bf,
                                    isOutput=False)
    wqp = nc.declare_dram_parameter("wqp", [P, KC * DQ], bf, isOutput=False)
    wkp = nc.declare_dram_parameter("wkp", [P, KC * DQ], bf, isOutput=False)
    wvp = nc.declare_dram_parameter("wvp", [P, KC * DQ], bf, isOutput=False)
    wop = nc.declare_dram_parameter("wop", [P, KC * DQ], bf, isOutput=False)
    cro = nc.declare_dram_parameter("cro", [P, S], bf, isOutput=False)
    sro = nc.declare_dram_parameter("sro", [P, S], bf, isOutput=False)
    cst = nc.declare_dram_parameter("cst", [P, 2 * P], bf, isOutput=False)
    mskT = None
    if mask_mode == "general":
        mskT = nc.declare_dram_parameter("mskT", [S, S], bf, isOutput=False)
    outT = nc.declare_dram_parameter("outT", [DQ, NT], f32, isOutput=True)
    attnD = agD = qD = kD = vD = None
    if dbg:
        attnD = nc.declare_dram_parameter("attnD", [P, B * SBK * HPC * HD],
                                          bf, isOutput=True)
        agD = nc.declare_dram_parameter("agD", [N_CORES * 2 * SH, HD], bf,
                                        isOutput=True)
        qD = nc.declare_dram_parameter("qD", [P, HPC * NT], bf, isOutput=True)
        kD = nc.declare_dram_parameter("kD", [P, HPC * NT], bf, isOutput=True)
        vD = nc.declare_dram_parameter("vD", [P, B * SBK * HPC * (HD + 1)],
                                       bf, isOutput=True)

    rg = [list(range(N_CORES))]

    with tile.TileContext(nc) as tc:
        with (
            tc.tile_pool(name="per", bufs=1) as per,
            tc.tile_pool(name="stage", bufs=4) as stage,
            tc.tile_pool(name="rt", bufs=3) as rt,
            tc.tile_pool(name="dram", bufs=1, space="DRAM") as drp,
            tc.tile_pool(name="ptp",
                         bufs=(1 if mask_mode == "general" else 2)) as ptp,
            tc.tile_pool(name="xs", bufs=3) as xs,
            tc.tile_pool(name="ags", bufs=8) as ags,
            tc.tile_pool(name="ostp", bufs=4) as ostp,
            tc.tile_pool(name="mkp", bufs=4) as mkp,
            tc.tile_pool(name="ps_qk", bufs=2, space="PSUM") as ps_qk,
            tc.tile_pool(name="ps_v", bufs=2, space="PSUM") as ps_v,
            tc.tile_pool(name="ps_st", bufs=2, space="PSUM") as ps_st,
            tc.tile_pool(name="ps_pv", bufs=2, space="PSUM") as ps_pv,
        ):
            # ---------------- persistent SBUF ----------------
            q_sb = per.tile([P, HPC * NT], bf)       # d-major Q, head h at h*NT
            k_sb = per.tile([P, HPC * NT], bf)
            vaug_sb = per.tile([P, B * SBK * HPC * (HD + 1)], bf)
            attn_sb = per.tile([P, B * SBK * HPC * HD], bf)  # token-major out
            wo_sb = per.tile([P, KC * DQ], bf)
            cst_sb = per.tile([P, 2 * P], bf)
            perm = cst_sb[:, 0:P]
            tri01 = cst_sb[:, P:2 * P]

            wq_sb = per.tile([P, KC * DQ], bf, name="wq_sb")
            wk_sb = per.tile([P, KC * DQ], bf, name="wk_sb")
            wv_sb = per.tile([P, KC * DQ], bf, name="wv_sb")
            cro_sb = per.tile([P, S], bf, name="cro_sb")
            sro_sb = per.tile([P, S], bf, name="sro_sb")

            nc.sync.dma_start(out=cst_sb[:], in_=cst[:, :])
            nc.sync.dma_start(out=wo_sb[:], in_=wop[:, :])
            nc.scalar.dma_start(out=wq_sb[:], in_=wqp[:, :])
            nc.scalar.dma_start(out=wk_sb[:], in_=wkp[:, :])
            nc.scalar.dma_start(out=wv_sb[:], in_=wvp[:, :])
            nc.scalar.dma_start(out=cro_sb[:], in_=cro[:, :])
            nc.scalar.dma_start(out=sro_sb[:], in_=sro[:, :])
            # ones columns for the PV denominator trick
            nc.gpsimd.memset(vaug_sb[:], 1.0)

            # token-major h-major bounce: rows h*SH + t_local, cols dd
            bounce = [[drp.tile([HPC * SH, HD], bf, name=f"bounce{b}{f}",
                                tag=f"bounce{b}{f}")
                       for f in range(2)] for b in range(B)]
            ag = [[drp.tile([N_CORES * HPC * SH, HD], bf, addr_space="Shared",
                            name=f"ag{b}{f}", tag=f"ag{b}{f}")
                   for f in range(2)] for b in range(B)]

            def rope(ps, dst, t0b):
                # dst (bf16, [P, TCH]) = cos*z + sin*pairswap(z); tables are
                # pre-swizzled so this is cro*z + sro*zsw elementwise
                zb = stage.tile([P, TCH], bf, tag="zb", name="zb")
                nc.vector.tensor_copy(zb[:], ps[:])
                # pairswap via PE permutation matmul (DVE strided-copy swap
                # mis-executes in this kernel despite passing in isolation)
                zs = ps_st.tile([P, TCH], f32, tag="st", name="zs")
                nc.tensor.matmul(zs[:], perm, zb[:])
                t1 = rt.tile([P, TCH], f32, tag="t1", name="t1")
                t2 = rt.tile([P, TCH], f32, tag="t2", name="t2")
                nc.vector.tensor_mul(t1[:], zb[:], cro_sb[:, t0b:t0b + TCH])
                nc.vector.tensor_mul(t2[:], zs[:], sro_sb[:, t0b:t0b + TCH])
                nc.vector.tensor_add(dst, t1[:], t2[:])

            def attn_chunk(b, qc):
                # attention for q tokens [qc*512, (qc+1)*512) of batch b
                for h in range(HPC):
                    qoff = h * NT + b * S
                    n_s = SBK if not causal else 4 * qc + 4
                    pt = ptp.tile([P, SBK * QCH], bf, tag="pt",
                                  name=f"pt{b}{h}{qc}")
                    for sb in range(n_s):
                        stp = ps_st.tile([P, QCH], f32, tag="st",
                                         name=f"st{b}{h}{qc}{sb}")
                        nc.tensor.matmul(
                            stp[:],
                            k_sb[:, qoff + sb * P:qoff + (sb + 1) * P],
                            q_sb[:, qoff + qc * QCH:qoff + (qc + 1) * QCH])
                        if mask_mode == "general":
                            mk = mkp.tile([P, QCH], bf, tag="mk",
                                          name=f"mk{b}{h}{qc}{sb}")
                            nc.sync.dma_start(
                                out=mk[:],
                                in_=mskT[sb * P:(sb + 1) * P,
                                         qc * QCH:(qc + 1) * QCH])
                            nc.vector.tensor_add(stp[:], stp[:], mk[:])
                        off = (sb - 4 * qc) * P \
                            if (causal and not fullexp and sb > 4 * qc) else 0
                        nc.scalar.activation(
                            pt[:, sb * QCH + off:(sb + 1) * QCH],
                            stp[:, off:QCH], EXP, scale=scale)
                    if causal:
                        for j in range(QCH // P):
                            sb = 4 * qc + j
                            c0 = sb * QCH + j * P
                            nc.vector.tensor_mul(
                                pt[:, c0:c0 + P], pt[:, c0:c0 + P], tri01)
                    for jj in range(QCH // P):
                        qb = 4 * qc + jj
                        n_pv = SBK if not causal else qb + 1
                        pv = ps_pv.tile([P, HD + 1], f32, tag="pv",
                                        name=f"pv{b}{h}{qb}")
                        for sb in range(n_pv):
                            nc.tensor.matmul(
                                pv[:],
                                pt[:, sb * QCH + jj * P:
                                   sb * QCH + (jj + 1) * P],
                                vaug_sb[:, _vaug_col(b, sb, h):
                                        _vaug_col(b, sb, h) + HD + 1],
                                start=(sb == 0), stop=(sb == n_pv - 1))
                        rec = stage.tile([P, 1], f32, tag="rec",
                                         name=f"rec{b}{h}{qb}")
                        nc.vector.reciprocal(rec[:], pv[:, HD:HD + 1])
                        nc.vector.tensor_scalar_mul(
                            attn_sb[:, _attn_col(b, qb, h):
                                    _attn_col(b, qb, h) + HD],
                            pv[:, 0:HD], rec[:])

            def flush_half(b, half):
                # token-major h-major bounce out, then AllGather
                for h in range(HPC):
                    for il in range(SH // P):
                        i = half * (SH // P) + il
                        nc.gpsimd.dma_start(
                            out=bounce[b][half][h * SH + il * P:
                                                h * SH + (il + 1) * P, :],
                            in_=attn_sb[:, _attn_col(b, i, h):
                                        _attn_col(b, i, h) + HD])
                nc.gpsimd.collective_compute(
                    "AllGather", mybir.AluOpType.bypass,
                    replica_groups=rg,
                    ins=[bounce[b][half].opt()], outs=[ag[b][half].opt()])

            def outproj(b, half):
                # NOTE: all X-bar transpose DMAs on ONE HWDGE queue (sync) —
                # concurrent transposes from two queues corrupt data on HW.
                agts = []
                for kk in range(KC):
                    agt = ags.tile([P, SH], bf, tag="agt",
                                   name=f"agt{b}{half}{kk}")
                    nc.sync.dma_start(
                        out=agt[:],
                        in_=ag[b][half][kk * SH:(kk + 1) * SH, :],
                        transpose=True)
                    agts.append(agt)
                for tg in range(SH // QCH):
                    pool = ps_qk if tg == 0 else ps_v
                    tag = "qkps" if tg == 0 else "vps"
                    ops = [pool.tile([P, QCH], f32, tag=tag,
                                     name=f"op{b}{half}{tg}{oc}")
                           for oc in range(2)]
                    for kk in range(KC):
                        for oc in range(2):
                            nc.tensor.matmul(
                                ops[oc],
                                wo_sb[:, kk * DQ + oc * P:
                                      kk * DQ + (oc + 1) * P],
                                agts[kk][:, tg * QCH:(tg + 1) * QCH],
                                start=(kk == 0), stop=(kk == KC - 1))
                    t0 = b * S + half * SH + tg * QCH
                    for oc in range(2):
                        ost = ostp.tile([P, QCH], f32, tag="ost",
                                        name=f"ost{b}{half}{tg}{oc}")
                        nc.vector.tensor_copy(ost[:], ops[oc][:])
                        nc.gpsimd.dma_start(
                            out=outT[oc * P:(oc + 1) * P, t0:t0 + QCH],
                            in_=ost[:])

            # ---------------- main pipeline ----------------
            for tci in range(NTC):
                t0 = tci * TCH
                b = tci // (NTC // B)
                lc = tci % (NTC // B)      # chunk index within batch
                t0b = lc * TCH             # within-batch token offset
                x_t = []
                for xh in range(2):
                    xt_h = xs.tile([P, KC * TCH // 2], bf, tag="xt",
                                   name=f"xt{tci}_{xh}")
                    nc.sync.dma_start(
                        out=xt_h[:],
                        in_=xRe[:, (tci * KC + xh * KC // 2) * TCH:
                                (tci * KC + (xh + 1) * KC // 2) * TCH])
                    x_t.append(xt_h)

                vp = [ps_v.tile([P, 2 * DQ], f32, tag="vps",
                                name=f"vp{tci}_{u}") for u in range(2)]
                for m in range(HPC):
                    qp = ps_qk.tile([P, TCH], f32, tag="qkps",
                                    name=f"qp{tci}_{m}")
                    kp = ps_qk.tile([P, TCH], f32, tag="qkps",
                                    name=f"kp{tci}_{m}")
                    vfirst = {}
                    for kk in range(KC):
                        xth = x_t[kk // (KC // 2)]
                        kkl = kk % (KC // 2)
                        xt = xth[:, kkl * TCH:(kkl + 1) * TCH]
                        st = (kk == 0)
                        sp = (kk == KC - 1)
                        nc.tensor.matmul(
                            qp[:],
                            wq_sb[:, kk * DQ + m * HD:kk * DQ + (m + 1) * HD],
                            xt, start=st, stop=sp)
                        nc.tensor.matmul(
                            kp[:],
                            wk_sb[:, kk * DQ + m * HD:kk * DQ + (m + 1) * HD],
                            xt, start=st, stop=sp)
                        if m == 0:
                            for tb in range(TCH // P):
                                mm = nc.tensor.matmul(
                                    vp[tb // 2][:, (tb % 2) * DQ:
                                                (tb % 2 + 1) * DQ],
                                    xth[:, kkl * TCH + tb * P:
                                        kkl * TCH + (tb + 1) * P],
                                    wv_sb[:, kk * DQ:(kk + 1) * DQ],
                                    start=(st and tb % 2 == 0), stop=sp,
                                    skip_group_check=(tb % 2 == 1))
                                if kk == 0:
                                    vfirst[tb] = mm
                    if m == 0:
                        for u in range(2):
                            add_dep_helper(vfirst[u * 2 + 1].ins,
                                           vfirst[u * 2].ins, sync=False,
                                           reason="bank-clear 2nd V group")
                    rope(qp, q_sb[:, m * NT + t0:m * NT + t0 + TCH], t0b)
                    rope(kp, k_sb[:, m * NT + t0:m * NT + t0 + TCH], t0b)
                # V psum -> vaug (token-major, per head)
                for tb in range(TCH // P):
                    i = (t0b + tb * P) // P
                    for h in range(HPC):
                        c0 = _vaug_col(b, i, h)
                        nc.vector.tensor_copy(
                            vaug_sb[:, c0:c0 + HD],
                            vp[tb // 2][:, (tb % 2) * DQ + h * HD:
                                        (tb % 2) * DQ + (h + 1) * HD])

                if causal:
                    attn_chunk(b, lc)
                    if lc % 2 == 1:
                        flush_half(b, lc // 2)
                elif lc == (NTC // B) - 1:
                    # non-causal needs the batch's full K/V first
                    for qc in range(4):
                        attn_chunk(b, qc)
                    flush_half(b, 0)
                    flush_half(b, 1)

            outproj(0, 0)
            outproj(0, 1)
            outproj(1, 0)
            outproj(1, 1)
            if dbg:
                nc.sync.dma_start(out=attnD[:, :], in_=attn_sb[:])
                nc.gpsimd.dma_start(out=agD[:, :], in_=ag[0][1][:, :])
                nc.sync.dma_start(out=qD[:, :], in_=q_sb[:])
                nc.sync.dma_start(out=kD[:, :], in_=k_sb[:])
                nc.sync.dma_start(out=vD[:, :], in_=vaug_sb[:])

    nc.compile()
    return nc


def _host_prep(inputs):
    x = np.ascontiguousarray(np.asarray(inputs["x"], np.float32).reshape(NT, D))
    wq = np.asarray(inputs["wq"], np.float32)
    wk = np.asarray(inputs["wk"], np.float32)
    wv = np.asarray(inputs["wv"], np.float32)
    wo = np.asarray(inputs["wo"], np.float32)
    cos = np.asarray(inputs["freqs_cos"], np.float32)
    sin = np.asarray(inputs["freqs_sin"], np.float32)
    mask = np.asarray(inputs["mask"], np.float32).reshape(S, S)

    tril = np.tril(np.ones((S, S), bool))
    if not mask.any():
        mode = "zeros"
    elif (mask[tril] == 0).all() and (mask[~tril] <= -1e8).all():
        mode = "causal"
    else:
        mode = "general"

    # x packed tci-major: xRe[p, (tci*KC+kk)*TCH + t] = x[tci*TCH+t, kk*P+p]
    xRe = np.ascontiguousarray(
        x.reshape(NTC, TCH, KC, P).transpose(3, 0, 2, 1)
        .reshape(P, NTC * KC * TCH).astype(BF))

    C = np.empty((P, S), np.float32)
    Sn = np.empty((P, S), np.float32)
    C[0::2] = cos.T
    C[1::2] = cos.T
    Sn[0::2] = -sin.T
    Sn[1::2] = sin.T
    cro = np.ascontiguousarray(C.astype(BF))
    sro = np.ascontiguousarray(Sn.astype(BF))
    cst = np.zeros((P, 2 * P), np.float32)
    pr = np.zeros((P, P), np.float32)
    idx = np.arange(0, P, 2)
    pr[idx, idx + 1] = 1.0
    pr[idx + 1, idx] = 1.0
    cst[:, 0:P] = pr
    cst[:, P:2 * P] = np.triu(np.ones((P, P), np.float32))
    cst = np.ascontiguousarray(cst.astype(BF))

    def wpack(w, r):
        wT = np.ascontiguousarray(w[r, :].T)
        return np.ascontiguousarray(
            wT.reshape(KC, P, DQ).transpose(1, 0, 2)
            .reshape(P, KC * DQ).astype(BF))

    in_maps = []
    for c in range(N_CORES):
        r = slice(c * DQ, (c + 1) * DQ)
        m = {
            "xRe": xRe,
            "wqp": wpack(wq, r),
            "wkp": wpack(wk, r),
            "wvp": wpack(wv, r),
            "wop": wpack(wo, r),
            "cro": cro,
            "sro": sro,
            "cst": cst,
        }
        if mode == "general":
            m["mskT"] = np.ascontiguousarray(
                (mask.T * math.sqrt(HD)).astype(BF))
        in_maps.append(m)
    return mode, in_maps


LAST_RESULT = None


def kernel(**inputs):
    global LAST_RESULT
    from concourse.bass_utils import run_bass_kernel_spmd

    mode, in_maps = _host_prep(inputs)
    if mode not in _cache:
        _cache[mode] = _build(mode)
    nc = _cache[mode]

    res = run_bass_kernel_spmd(nc, in_maps, list(range(N_CORES)))
    LAST_RESULT = res

    out_full = np.empty((NT, D), np.float32)
    for c in range(N_CORES):
        out_full[:, c * DQ:(c + 1) * DQ] = res.results[c]["outT"].T
    return out_full.reshape(B, S, D)


# revision 37
# speedup vs baseline: 1.1735x; 1.0906x over previous
"""Multi-head causal attention (QKV proj + RoPE + softmax attention + out proj)
as a distributed Bass kernel on 8 Trainium2 NeuronCores.

Sharding: tensor-parallel over heads (2 of 16 heads per core).

Pipeline (v3): one token chunk (512) per step — QKV projection + RoPE for
the chunk, then immediately the attention q-chunk it enables (causal), so
the 4 AllGathers fire evenly through the kernel instead of bunching at the
end. Everything lives in ONE tile-pool scope (no mid-kernel pool barrier).

Layouts: Q/K d-major in SBUF; V token-major with a ones column (softmax
denominator comes free out of the PV matmul); attention output token-major,
normalized in a single DVE op; AllGather is token-major h-major rows so the
out-projection can read it back d-major via contiguous X-bar DMA-transposes
(all on ONE HWDGE queue — concurrent transposes from two queues corrupt).
Out-projection runs flipped (wo stationary, tokens moving, N=512) emitting
out^T; the host transposes and concatenates the 8 column slices.

All matmuls bf16 (fp32 PSUM); softmax without max-subtraction (scores ~N(0,1)).
"""

import math
import os as _os
import numpy as np
import ml_dtypes

B, S, D, H = 2, 2048, 2048, 16
HD = 128                  # head dim
P = 128                   # SBUF partitions
NT = B * S                # 4096 tokens
N_CORES = 8
HPC = H // N_CORES        # heads per core
DQ = HPC * HD             # 256 q/k/v rows per core
KC = D // P               # 16 contraction chunks
TCH = 512                 # token chunk
NTC = NT // TCH           # 8
SBK = S // P              # 16 key blocks per batch
QCH = 512                 # q chunk in attention
SH = S // 2               # AllGather half (1024 tokens)
BF = ml_dtypes.bfloat16

_cache = {}


def _vaug_col(b, i, h):
    return ((b * SBK + i) * HPC + h) * (HD + 1)


def _attn_col(b, i, h):
    return ((b * SBK + i) * HPC + h) * HD


def _build(mask_mode):
    from concourse import bacc
    import concourse.mybir as mybir
    import concourse.tile as tile
    from concourse.tile_rust import add_dep_helper

    bf = mybir.dt.bfloat16
    f32 = mybir.dt.float32
    EXP = mybir.ActivationFunctionType.Exp
    scale = 1.0 / math.sqrt(HD)
    causal = mask_mode == "causal"
    fullexp = _os.environ.get("KFULLEXP", "0") == "1"
    dbg = _os.environ.get("KDBG", "0") == "1"

    nc = bacc.Bacc("TRN2", target_bir_lowering=False, debug=False,
                   num_devices=N_CORES)

    xRe = nc.declare_dram_parameter("xRe", [P, NTC * KC * TCH], bf,
                                    isOutput=False)
    wqp = nc.declare_dram_parameter("wqp", [P, KC * DQ], bf, isOutput=False)
    wkp = nc.declare_dram_parameter("wkp", [P, KC * DQ], bf, isOutput=False)
    wvp = nc.declare_dram_parameter("wvp", [P, KC * DQ], bf, isOutput=False)
    wop = nc.declare_dram_parameter("wop", [P, KC * DQ], bf, isOutput=False)
    cro = nc.declare_dram_parameter("cro", [P, S], bf, isOutput=False)
    sro = nc.declare_dram_parameter("sro", [P, S], bf, isOutput=False)
    cst = nc.declare_dram_parameter("cst", [P, 2 * P], bf, isOutput=False)
    mskT = None
    if mask_mode == "general":
        mskT = nc.declare_dram_parameter("mskT", [S, S], bf, isOutput=False)
    outT = nc.declare_dram_parameter("outT", [DQ, NT], f32, isOutput=True)
    attnD = agD = qD = kD = vD = None
    if dbg:
        attnD = nc.declare_dram_parameter("attnD", [P, B * SBK * HPC * HD],
                                          bf, isOutput=True)
        agD = nc.declare_dram_parameter("agD", [N_CORES * 2 * SH, HD], bf,
                                        isOutput=True)
        qD = nc.declare_dram_parameter("qD", [P, HPC * NT], bf, isOutput=True)
        kD = nc.declare_dram_parameter("kD", [P, HPC * NT], bf, isOutput=True)
        vD = nc.declare_dram_parameter("vD", [P, B * SBK * HPC * (HD + 1)],
                                       bf, isOutput=True)

    rg = [list(range(N_CORES))]

    with tile.TileContext(nc) as tc:
        with (
            tc.tile_pool(name="per", bufs=1) as per,
            tc.tile_pool(name="stage", bufs=3) as stage,
            tc.tile_pool(name="rt", bufs=2) as rt,
            tc.tile_pool(name="dram", bufs=1, space="DRAM") as drp,
            tc.tile_pool(name="ptp",
                         bufs=(1 if mask_mode == "general" else 2)) as ptp,
            tc.tile_pool(name="xs", bufs=5) as xs,
            tc.tile_pool(name="ags", bufs=4) as ags,
            tc.tile_pool(name="ostp", bufs=2) as ostp,
            tc.tile_pool(name="mkp", bufs=4) as mkp,
            tc.tile_pool(name="ps_qk", bufs=2, space="PSUM") as ps_qk,
            tc.tile_pool(name="ps_v", bufs=2, space="PSUM") as ps_v,
            tc.tile_pool(name="ps_st", bufs=2, space="PSUM") as ps_st,
            tc.tile_pool(name="ps_pv", bufs=2, space="PSUM") as ps_pv,
        ):
            # ---------------- persistent SBUF ----------------
            q_sb = per.tile([P, HPC * NT], bf)       # d-major Q, head h at h*NT
            k_sb = per.tile([P, HPC * NT], bf)
            vaug_sb = per.tile([P, B * SBK * HPC * (HD + 1)], bf)
            attn_sb = per.tile([P, B * SBK * HPC * HD], bf)  # token-major out
            wo_sb = per.tile([P, KC * DQ], bf)
            cst_sb = per.tile([P, 2 * P], bf)
            perm = cst_sb[:, 0:P]
            tri01 = cst_sb[:, P:2 * P]

            wq_sb = per.tile([P, KC * DQ], bf, name="wq_sb")
            wk_sb = per.tile([P, KC * DQ], bf, name="wk_sb")
            wv_sb = per.tile([P, KC * DQ], bf, name="wv_sb")
            cro_sb = per.tile([P, S], bf, name="cro_sb")
            sro_sb = per.tile([P, S], bf, name="sro_sb")

            nc.sync.dma_start(out=cst_sb[:], in_=cst[:, :])
            nc.sync.dma_start(out=wo_sb[:], in_=wop[:, :])
            nc.scalar.dma_start(out=wq_sb[:], in_=wqp[:, :])
            nc.scalar.dma_start(out=wk_sb[:], in_=wkp[:, :])
            nc.scalar.dma_start(out=wv_sb[:], in_=wvp[:, :])
            nc.scalar.dma_start(out=cro_sb[:], in_=cro[:, :])
            nc.scalar.dma_start(out=sro_sb[:], in_=sro[:, :])
            # ones columns for the PV denominator trick
            nc.gpsimd.memset(vaug_sb[:], 1.0)

            # token-major h-major bounce: rows h*SH + t_local, cols dd
            bounce = [[drp.tile([HPC * SH, HD], bf, name=f"bounce{b}{f}",
                                tag=f"bounce{b}{f}")
                       for f in range(2)] for b in range(B)]
            ag = [[drp.tile([N_CORES * HPC * SH, HD], bf, addr_space="Shared",
                            name=f"ag{b}{f}", tag=f"ag{b}{f}")
                   for f in range(2)] for b in range(B)]

            def rope(ps, dst, t0b):
                # dst (bf16, [P, TCH]) = cos*z + sin*pairswap(z); tables are
                # pre-swizzled so this is cro*z + sro*zsw elementwise
                zb = stage.tile([P, TCH], bf, tag="zb", name="zb")
                nc.vector.tensor_copy(zb[:], ps[:])
                # pairswap via PE permutation matmul (DVE strided-copy swap
                # mis-executes in this kernel despite passing in isolation)
                zs = ps_st.tile([P, TCH], f32, tag="st", name="zs")
                nc.tensor.matmul(zs[:], perm, zb[:])
                t1 = rt.tile([P, TCH], f32, tag="t1", name="t1")
                t2 = rt.tile([P, TCH], f32, tag="t2", name="t2")
                nc.vector.tensor_mul(t1[:], zb[:], cro_sb[:, t0b:t0b + TCH])
                nc.vector.tensor_mul(t2[:], zs[:], sro_sb[:, t0b:t0b + TCH])
                nc.vector.tensor_add(dst, t1[:], t2[:])

            def attn_chunk(b, qc):
                # attention for q tokens [qc*512, (qc+1)*512) of batch b
                for h in range(HPC):
                    qoff = h * NT + b * S
                    n_s = SBK if not causal else 4 * qc + 4
                    pt = ptp.tile([P, SBK * QCH], bf, tag="pt",
                                  name=f"pt{b}{h}{qc}")
                    for sb in range(n_s):
                        stp = ps_st.tile([P, QCH], f32, tag="st",
                                         name=f"st{b}{h}{qc}{sb}")
                        nc.tensor.matmul(
                            stp[:],
                            k_sb[:, qoff + sb * P:qoff + (sb + 1) * P],
                            q_sb[:, qoff + qc * QCH:qoff + (qc + 1) * QCH])
                        if mask_mode == "general":
                            mk = mkp.tile([P, QCH], bf, tag="mk",
                                          name=f"mk{b}{h}{qc}{sb}")
                            nc.sync.dma_start(
                                out=mk[:],
                                in_=mskT[sb * P:(sb + 1) * P,
                                         qc * QCH:(qc + 1) * QCH])
                            nc.vector.tensor_add(stp[:], stp[:], mk[:])
                        off = (sb - 4 * qc) * P \
                            if (causal and not fullexp and sb > 4 * qc) else 0
                        nc.scalar.activation(
                            pt[:, sb * QCH + off:(sb + 1) * QCH],
                            stp[:, off:QCH], EXP, scale=scale)
                    if causal:
                        for j in range(QCH // P):
                            sb = 4 * qc + j
                            c0 = sb * QCH + j * P
                            nc.vector.tensor_mul(
                                pt[:, c0:c0 + P], pt[:, c0:c0 + P], tri01)
                    for jj in range(QCH // P):
                        qb = 4 * qc + jj
                        n_pv = SBK if not causal else qb + 1
                        pv = ps_pv.tile([P, HD + 1], f32, tag="pv",
                                        name=f"pv{b}{h}{qb}")
                        for sb in range(n_pv):
                            nc.tensor.matmul(
                                pv[:],
                                pt[:, sb * QCH + jj * P:
                                   sb * QCH + (jj + 1) * P],
                                vaug_sb[:, _vaug_col(b, sb, h):
                                        _vaug_col(b, sb, h) + HD + 1],
                                start=(sb == 0), stop=(sb == n_pv - 1))
                        rec = stage.tile([P, 1], f32, tag="rec",
                                         name=f"rec{b}{h}{qb}")
                        nc.vector.reciprocal(rec[:], pv[:, HD:HD + 1])
                        nc.vector.tensor_scalar_mul(
                            attn_sb[:, _attn_col(b, qb, h):
                                    _attn_col(b, qb, h) + HD],
                            pv[:, 0:HD], rec[:])

            def flush_half(b, half):
                # token-major h-major bounce out, then AllGather
                for h in range(HPC):
                    for il in range(SH // P):
                        i = half * (SH // P) + il
                        nc.gpsimd.dma_start(
                            out=bounce[b][half][h * SH + il * P:
                                                h * SH + (il + 1) * P, :],
                            in_=attn_sb[:, _attn_col(b, i, h):
                                        _attn_col(b, i, h) + HD])
                nc.gpsimd.collective_compute(
                    "AllGather", mybir.AluOpType.bypass,
                    replica_groups=rg,
                    ins=[bounce[b][half].opt()], outs=[ag[b][half].opt()])

            def outproj(b, half):
                # NOTE: all X-bar transpose DMAs on ONE HWDGE queue (sync) —
                # concurrent transposes from two queues corrupt data on HW.
                # 4 quarter-transposes: [4*SH, HD] -> [HD, 4*SH] covers 4 kk
                agts = []
                for q in range(4):
                    agt = ags.tile([P, 4 * SH], bf, tag="agt",
                                   name=f"agt{b}{half}{q}")
                    nc.sync.dma_start(
                        out=agt[:],
                        in_=ag[b][half][q * 4 * SH:(q + 1) * 4 * SH, :],
                        transpose=True)
                    agts.append(agt)
                for tg in range(SH // QCH):
                    pool = ps_qk if tg == 0 else ps_v
                    tag = "qkps" if tg == 0 else "vps"
                    ops = [pool.tile([P, QCH], f32, tag=tag,
                                     name=f"op{b}{half}{tg}{oc}")
                           for oc in range(2)]
                    for kk in range(KC):
                        mv = agts[kk // 4][:, (kk % 4) * SH + tg * QCH:
                                          (kk % 4) * SH + (tg + 1) * QCH]
                        for oc in range(2):
                            nc.tensor.matmul(
                                ops[oc],
                                wo_sb[:, kk * DQ + oc * P:
                                      kk * DQ + (oc + 1) * P],
                                mv,
                                start=(kk == 0), stop=(kk == KC - 1))
                    t0 = b * S + half * SH + tg * QCH
                    for oc in range(2):
                        ost = ostp.tile([P, QCH], f32, tag="ost",
                                        name=f"ost{b}{half}{tg}{oc}")
                        nc.vector.tensor_copy(ost[:], ops[oc][:])
                        nc.gpsimd.dma_start(
                            out=outT[oc * P:(oc + 1) * P, t0:t0 + QCH],
                            in_=ost[:])

            # ---------------- main pipeline ----------------
            for tci in range(NTC):
                t0 = tci * TCH
                b = tci // (NTC // B)
                lc = tci % (NTC // B)      # chunk index within batch
                t0b = lc * TCH             # within-batch token offset
                x_t = []
                for xh in range(4):
                    xt_h = xs.tile([P, KC * TCH // 4], bf, tag="xt",
                                   name=f"xt{tci}_{xh}")
                    nc.scalar.dma_start(
                        out=xt_h[:],
                        in_=xRe[:, (tci * KC + xh * KC // 4) * TCH:
                                (tci * KC + (xh + 1) * KC // 4) * TCH])
                    x_t.append(xt_h)

                vp = [ps_v.tile([P, 2 * DQ], f32, tag="vps",
                                name=f"vp{tci}_{u}") for u in range(2)]
                for m in range(HPC):
                    qp = ps_qk.tile([P, TCH], f32, tag="qkps",
                                    name=f"qp{tci}_{m}")
                    kp = ps_qk.tile([P, TCH], f32, tag="qkps",
                                    name=f"kp{tci}_{m}")
                    vfirst = {}
                    for kk in range(KC):
                        xth = x_t[kk // (KC // 4)]
                        kkl = kk % (KC // 4)
                        xt = xth[:, kkl * TCH:(kkl + 1) * TCH]
                        st = (kk == 0)
                        sp = (kk == KC - 1)
                        nc.tensor.matmul(
                            qp[:],
                            wq_sb[:, kk * DQ + m * HD:kk * DQ + (m + 1) * HD],
                            xt, start=st, stop=sp)
                        nc.tensor.matmul(
                            kp[:],
                            wk_sb[:, kk * DQ + m * HD:kk * DQ + (m + 1) * HD],
                            xt, start=st, stop=sp)
                        if m == 0:
                            for tb in range(TCH // P):
                                mm = nc.tensor.matmul(
                                    vp[tb // 2][:, (tb % 2) * DQ:
                                                (tb % 2 + 1) * DQ],
                                    xth[:, kkl * TCH + tb * P:
                                        kkl * TCH + (tb + 1) * P],
                                    wv_sb[:, kk * DQ:(kk + 1) * DQ],
                                    start=(st and tb % 2 == 0), stop=sp,
                                    skip_group_check=(tb % 2 == 1))
                                if kk == 0:
                                    vfirst[tb] = mm
                    if m == 0:
                        for u in range(2):
                            add_dep_helper(vfirst[u * 2 + 1].ins,
                                           vfirst[u * 2].ins, sync=False,
                                           reason="bank-clear 2nd V group")
                    rope(qp, q_sb[:, m * NT + t0:m * NT + t0 + TCH], t0b)
                    rope(kp, k_sb[:, m * NT + t0:m * NT + t0 + TCH], t0b)
                # V psum -> vaug (token-major, per head)
                for tb in range(TCH // P):
                    i = (t0b + tb * P) // P
                    for h in range(HPC):
                        c0 = _vaug_col(b, i, h)
                        nc.vector.tensor_copy(
                            vaug_sb[:, c0:c0 + HD],
                            vp[tb // 2][:, (tb % 2) * DQ + h * HD:
                                        (tb % 2) * DQ + (h + 1) * HD])

                if causal:
                    attn_chunk(b, lc)
                    if lc % 2 == 1:
                        flush_half(b, lc // 2)
                elif lc == (NTC // B) - 1:
                    # non-causal needs the batch's full K/V first
                    for qc in range(4):
                        attn_chunk(b, qc)
                    flush_half(b, 0)
                    flush_half(b, 1)

            outproj(0, 0)
            outproj(0, 1)
            outproj(1, 0)
            outproj(1, 1)
            if dbg:
                nc.sync.dma_start(out=attnD[:, :], in_=attn_sb[:])
                nc.gpsimd.dma_start(out=agD[:, :], in_=ag[0][1][:, :])
                nc.sync.dma_start(out=qD[:, :], in_=q_sb[:])
                nc.sync.dma_start(out=kD[:, :], in_=k_sb[:])
                nc.sync.dma_start(out=vD[:, :], in_=vaug_sb[:])

    nc.compile()
    return nc


def _host_prep(inputs):
    x = np.ascontiguousarray(np.asarray(inputs["x"], np.float32).reshape(NT, D))
    wq = np.asarray(inputs["wq"], np.float32)
    wk = np.asarray(inputs["wk"], np.float32)
    wv = np.asarray(inputs["wv"], np.float32)
    wo = np.asarray(inputs["wo"], np.float32)
    cos = np.asarray(inputs["freqs_cos"], np.float32)
    sin = np.asarray(inputs["freqs_sin"], np.float32)
    mask = np.asarray(inputs["mask"], np.float32).reshape(S, S)

    tril = np.tril(np.ones((S, S), bool))
    if not mask.any():
        mode = "zeros"
    elif (mask[tril] == 0).all() and (mask[~tril] <= -1e8).all():
        mode = "causal"
    else:
        mode = "general"

    # x packed tci-major: xRe[p, (tci*KC+kk)*TCH + t] = x[tci*TCH+t, kk*P+p]
    xRe = np.ascontiguousarray(
        x.reshape(NTC, TCH, KC, P).transpose(3, 0, 2, 1)
        .reshape(P, NTC * KC * TCH).astype(BF))

    C = np.empty((P, S), np.float32)
    Sn = np.empty((P, S), np.float32)
    C[0::2] = cos.T
    C[1::2] = cos.T
    Sn[0::2] = -sin.T
    Sn[1::2] = sin.T
    cro = np.ascontiguousarray(C.astype(BF))
    sro = np.ascontiguousarray(Sn.astype(BF))
    cst = np.zeros((P, 2 * P), np.float32)
    pr = np.zeros((P, P), np.float32)
    idx = np.arange(0, P, 2)
    pr[idx, idx + 1] = 1.0
    pr[idx + 1, idx] = 1.0
    cst[:, 0:P] = pr
    cst[:, P:2 * P] = np.triu(np.ones((P, P), np.float32))
    cst = np.ascontiguousarray(cst.astype(BF))

    def wpack(w, r):
        wT = np.ascontiguousarray(w[r, :].T)
        return np.ascontiguousarray(
            wT.reshape(KC, P, DQ).transpose(1, 0, 2)
            .reshape(P, KC * DQ).astype(BF))

    in_maps = []
    for c in range(N_CORES):
        r = slice(c * DQ, (c + 1) * DQ)
        m = {
            "xRe": xRe,
            "wqp": wpack(wq, r),
            "wkp": wpack(wk, r),
            "wvp": wpack(wv, r),
            "wop": wpack(wo, r),
            "cro": cro,
            "sro": sro,
            "cst": cst,
        }
        if mode == "general":
            m["mskT"] = np.ascontiguousarray(
                (mask.T * math.sqrt(HD)).astype(BF))
        in_maps.append(m)
    return mode, in_maps


LAST_RESULT = None


def kernel(**inputs):
    global LAST_RESULT
    from concourse.bass_utils import run_bass_kernel_spmd

    mode, in_maps = _host_prep(inputs)
    if mode not in _cache:
        _cache[mode] = _build(mode)
    nc = _cache[mode]

    res = run_bass_kernel_spmd(nc, in_maps, list(range(N_CORES)))
    LAST_RESULT = res

    out_full = np.empty((NT, D), np.float32)
    for c in range(N_CORES):
        out_full[:, c * DQ:(c + 1) * DQ] = res.results[c]["outT"].T
    return out_full.reshape(B, S, D)


# revision 47
# speedup vs baseline: 1.3614x; 1.1601x over previous
"""Multi-head causal attention (QKV proj + RoPE + softmax attention + out proj)
as a distributed Bass kernel on 8 Trainium2 NeuronCores.

Sharding: tensor-parallel over heads (2 of 16 heads per core).

Pipeline (v3): one token chunk (512) per step — QKV projection + RoPE for
the chunk, then immediately the attention q-chunk it enables (causal), so
the 4 AllGathers fire evenly through the kernel instead of bunching at the
end. Everything lives in ONE tile-pool scope (no mid-kernel pool barrier).

Layouts: Q/K d-major in SBUF; V token-major with a ones column (softmax
denominator comes free out of the PV matmul); attention output token-major,
normalized in a single DVE op; AllGather is token-major h-major rows so the
out-projection can read it back d-major via contiguous X-bar DMA-transposes
(all on ONE HWDGE queue — concurrent transposes from two queues corrupt).
Out-projection runs flipped (wo stationary, tokens moving, N=512) emitting
out^T; the host transposes and concatenates the 8 column slices.

All matmuls bf16 (fp32 PSUM); softmax without max-subtraction (scores ~N(0,1)).
"""

import math
import os as _os
import numpy as np
import ml_dtypes

B, S, D, H = 2, 2048, 2048, 16
HD = 128                  # head dim
P = 128                   # SBUF partitions
NT = B * S                # 4096 tokens
N_CORES = 8
HPC = H // N_CORES        # heads per core
DQ = HPC * HD             # 256 q/k/v rows per core
KC = D // P               # 16 contraction chunks
TCH = 512                 # token chunk
NTC = NT // TCH           # 8
SBK = S // P              # 16 key blocks per batch
QCH = 512                 # q chunk in attention
SH = S // 2               # AllGather half (1024 tokens)
BF = ml_dtypes.bfloat16

_cache = {}


def _vaug_col(b, i, h):
    return ((b * SBK + i) * HPC + h) * (HD + 1)


def _attn_col(b, i, h):
    return ((b * SBK + i) * HPC + h) * HD


def _build(mask_mode):
    from concourse import bacc
    import concourse.mybir as mybir
    import concourse.tile as tile
    from concourse.tile_rust import add_dep_helper

    bf = mybir.dt.bfloat16
    f32 = mybir.dt.float32
    EXP = mybir.ActivationFunctionType.Exp
    scale = 1.0 / math.sqrt(HD)
    causal = mask_mode == "causal"
    fullexp = _os.environ.get("KFULLEXP", "0") == "1"
    dbg = _os.environ.get("KDBG", "0") == "1"

    nc = bacc.Bacc("TRN2", target_bir_lowering=False, debug=False,
                   num_devices=N_CORES)

    xRe = nc.declare_dram_parameter("xRe", [P, NTC * KC * TCH], bf,
                                    isOutput=False)
    wqp = nc.declare_dram_parameter("wqp", [P, KC * DQ], bf, isOutput=False)
    wkp = nc.declare_dram_parameter("wkp", [P, KC * DQ], bf, isOutput=False)
    wvp = nc.declare_dram_parameter("wvp", [P, KC * DQ], bf, isOutput=False)
    wop = nc.declare_dram_parameter("wop", [P, KC * DQ], bf, isOutput=False)
    cro = nc.declare_dram_parameter("cro", [P, S], bf, isOutput=False)
    sro = nc.declare_dram_parameter("sro", [P, S], bf, isOutput=False)
    cst = nc.declare_dram_parameter("cst", [P, 3 * P], bf, isOutput=False)
    mskT = None
    if mask_mode == "general":
        mskT = nc.declare_dram_parameter("mskT", [S, S], bf, isOutput=False)
    outT = nc.declare_dram_parameter("outT", [DQ, NT], f32, isOutput=True)
    attnD = agD = qD = kD = vD = None
    if dbg:
        attnD = nc.declare_dram_parameter("attnD", [P, B * SBK * HPC * HD],
                                          bf, isOutput=True)
        agD = nc.declare_dram_parameter("agD", [N_CORES * DQ, SH], bf,
                                        isOutput=True)
        qD = nc.declare_dram_parameter("qD", [P, HPC * NT], bf, isOutput=True)
        kD = nc.declare_dram_parameter("kD", [P, HPC * NT], bf, isOutput=True)
        vD = nc.declare_dram_parameter("vD", [P, B * SBK * HPC * (HD + 1)],
                                       bf, isOutput=True)

    rg = [list(range(N_CORES))]

    with tile.TileContext(nc) as tc:
        with (
            tc.tile_pool(name="per", bufs=1) as per,
            tc.tile_pool(name="stage", bufs=3) as stage,
            tc.tile_pool(name="rt", bufs=2) as rt,
            tc.tile_pool(name="dram", bufs=1, space="DRAM") as drp,
            tc.tile_pool(name="ptp",
                         bufs=(1 if mask_mode == "general" else 2)) as ptp,
            tc.tile_pool(name="xs", bufs=5) as xs,
            tc.tile_pool(name="ags", bufs=12) as ags,
            tc.tile_pool(name="ostp", bufs=4) as ostp,
            tc.tile_pool(name="mkp", bufs=4) as mkp,
            tc.tile_pool(name="ps_qk", bufs=2, space="PSUM") as ps_qk,
            tc.tile_pool(name="ps_v", bufs=2, space="PSUM") as ps_v,
            tc.tile_pool(name="ps_st", bufs=2, space="PSUM") as ps_st,
            tc.tile_pool(name="ps_pv", bufs=2, space="PSUM") as ps_pv,
        ):
            # ---------------- persistent SBUF ----------------
            q_sb = per.tile([P, HPC * NT], bf)       # d-major Q, head h at h*NT
            k_sb = per.tile([P, HPC * NT], bf)
            vaug_sb = per.tile([P, B * SBK * HPC * (HD + 1)], bf)
            attn_sb = per.tile([P, B * SBK * HPC * HD], bf)  # token-major out
            wo_sb = per.tile([P, KC * DQ], bf)
            cst_sb = per.tile([P, 3 * P], bf)
            perm = cst_sb[:, 0:P]
            tri01 = cst_sb[:, P:2 * P]
            ident = cst_sb[:, 2 * P:3 * P]

            wq_sb = per.tile([P, KC * DQ], bf, name="wq_sb")
            wk_sb = per.tile([P, KC * DQ], bf, name="wk_sb")
            wv_sb = per.tile([P, KC * DQ], bf, name="wv_sb")
            cro_sb = per.tile([P, S], bf, name="cro_sb")
            sro_sb = per.tile([P, S], bf, name="sro_sb")

            nc.sync.dma_start(out=cst_sb[:], in_=cst[:, :])
            nc.sync.dma_start(out=wq_sb[:], in_=wqp[:, :])
            nc.scalar.dma_start(out=wk_sb[:], in_=wkp[:, :])
            nc.sync.dma_start(out=wv_sb[:], in_=wvp[:, :])
            nc.scalar.dma_start(out=cro_sb[:], in_=cro[:, :])
            nc.scalar.dma_start(out=sro_sb[:], in_=sro[:, :])
            nc.sync.dma_start(out=wo_sb[:], in_=wop[:, :])
            # ones columns for the PV denominator trick
            nc.gpsimd.memset(vaug_sb[:], 1.0)

            # d-major bounce: rows (h, dd), cols = tokens of the half
            bounce = [[drp.tile([DQ, SH], bf, name=f"bounce{b}{f}",
                                tag=f"bounce{b}{f}")
                       for f in range(2)] for b in range(B)]
            ag = [[drp.tile([N_CORES * DQ, SH], bf, addr_space="Shared",
                            name=f"ag{b}{f}", tag=f"ag{b}{f}")
                   for f in range(2)] for b in range(B)]

            def rope(ps, dst, t0b):
                # dst (bf16, [P, TCH]) = cos*z + sin*pairswap(z); tables are
                # pre-swizzled so this is cro*z + sro*zsw elementwise
                zb = stage.tile([P, TCH], bf, tag="zb", name="zb")
                nc.vector.tensor_copy(zb[:], ps[:])
                # pairswap via PE permutation matmul (DVE strided-copy swap
                # mis-executes in this kernel despite passing in isolation)
                zs = ps_st.tile([P, TCH], f32, tag="st", name="zs")
                nc.tensor.matmul(zs[:], perm, zb[:])
                t1 = rt.tile([P, TCH], f32, tag="t1", name="t1")
                t2 = rt.tile([P, TCH], f32, tag="t2", name="t2")
                nc.vector.tensor_mul(t1[:], zb[:], cro_sb[:, t0b:t0b + TCH])
                nc.vector.tensor_mul(t2[:], zs[:], sro_sb[:, t0b:t0b + TCH])
                nc.vector.tensor_add(dst, t1[:], t2[:])

            def attn_chunk(b, qc):
                # attention for q tokens [qc*512, (qc+1)*512) of batch b
                for h in range(HPC):
                    qoff = h * NT + b * S
                    n_s = SBK if not causal else 4 * qc + 4
                    pt = ptp.tile([P, SBK * QCH], bf, tag="pt",
                                  name=f"pt{b}{h}{qc}")
                    for sb in range(n_s):
                        stp = ps_st.tile([P, QCH], f32, tag="st",
                                         name=f"st{b}{h}{qc}{sb}")
                        nc.tensor.matmul(
                            stp[:],
                            k_sb[:, qoff + sb * P:qoff + (sb + 1) * P],
                            q_sb[:, qoff + qc * QCH:qoff + (qc + 1) * QCH])
                        if mask_mode == "general":
                            mk = mkp.tile([P, QCH], bf, tag="mk",
                                          name=f"mk{b}{h}{qc}{sb}")
                            nc.sync.dma_start(
                                out=mk[:],
                                in_=mskT[sb * P:(sb + 1) * P,
                                         qc * QCH:(qc + 1) * QCH])
                            nc.vector.tensor_add(stp[:], stp[:], mk[:])
                        off = (sb - 4 * qc) * P \
                            if (causal and not fullexp and sb > 4 * qc) else 0
                        nc.scalar.activation(
                            pt[:, sb * QCH + off:(sb + 1) * QCH],
                            stp[:, off:QCH], EXP, scale=scale)
                    if causal:
                        for j in range(QCH // P):
                            sb = 4 * qc + j
                            c0 = sb * QCH + j * P
                            nc.vector.tensor_mul(
                                pt[:, c0:c0 + P], pt[:, c0:c0 + P], tri01)
                    for jj in range(QCH // P):
                        qb = 4 * qc + jj
                        n_pv = SBK if not causal else qb + 1
                        pv = ps_pv.tile([P, HD + 1], f32, tag="pv",
                                        name=f"pv{b}{h}{qb}")
                        for sb in range(n_pv):
                            nc.tensor.matmul(
                                pv[:],
                                pt[:, sb * QCH + jj * P:
                                   sb * QCH + (jj + 1) * P],
                                vaug_sb[:, _vaug_col(b, sb, h):
                                        _vaug_col(b, sb, h) + HD + 1],
                                start=(sb == 0), stop=(sb == n_pv - 1))
                        rec = stage.tile([P, 1], f32, tag="rec",
                                         name=f"rec{b}{h}{qb}")
                        nc.vector.reciprocal(rec[:], pv[:, HD:HD + 1])
                        ast = stage.tile([P, P], bf, tag="ast",
                                         name=f"ast{b}{h}{qb}")
                        nc.vector.tensor_scalar_mul(ast[:], pv[:, 0:HD],
                                                    rec[:])
                        trp = ps_pv.tile([P, P], bf, tag="pv",
                                         name=f"tr{b}{h}{qb}")
                        nc.tensor.transpose(trp[:], ast[:], ident)
                        nc.vector.tensor_copy(
                            attn_sb[:, h * NT + b * S + qb * P:
                                    h * NT + b * S + (qb + 1) * P],
                            trp[:])

            def flush_half(b, half):
                # d-major bounce out, then AllGather
                for h in range(HPC):
                    c0 = h * NT + b * S + half * SH
                    nc.gpsimd.dma_start(
                        out=bounce[b][half][h * HD:(h + 1) * HD, :],
                        in_=attn_sb[:, c0:c0 + SH])
                nc.gpsimd.collective_compute(
                    "AllGather", mybir.AluOpType.bypass,
                    replica_groups=rg,
                    ins=[bounce[b][half].opt()], outs=[ag[b][half].opt()])

            def outproj(b, half):
                agts = []
                for kk in range(KC):
                    agt = ags.tile([P, SH], bf, tag="agt",
                                   name=f"agt{b}{half}{kk}")
                    nc.sync.dma_start(
                        out=agt[:],
                        in_=ag[b][half][kk * P:(kk + 1) * P, :])
                    agts.append(agt)
                for tg in range(SH // QCH):
                    pool = ps_qk if tg == 0 else ps_v
                    tag = "qkps" if tg == 0 else "vps"
                    ops = [pool.tile([P, QCH], f32, tag=tag,
                                     name=f"op{b}{half}{tg}{oc}")
                           for oc in range(2)]
                    for kk in range(KC):
                        mv = agts[kk][:, tg * QCH:(tg + 1) * QCH]
                        for oc in range(2):
                            nc.tensor.matmul(
                                ops[oc],
                                wo_sb[:, kk * DQ + oc * P:
                                      kk * DQ + (oc + 1) * P],
                                mv,
                                start=(kk == 0), stop=(kk == KC - 1))
                    t0 = b * S + half * SH + tg * QCH
                    for oc in range(2):
                        ost = ostp.tile([P, QCH], f32, tag="ost",
                                        name=f"ost{b}{half}{tg}{oc}")
                        nc.scalar.activation(
                            ost[:], ops[oc][:],
                            mybir.ActivationFunctionType.Copy)
                        nc.gpsimd.dma_start(
                            out=outT[oc * P:(oc + 1) * P, t0:t0 + QCH],
                            in_=ost[:])

            # ---------------- main pipeline ----------------
            for tci in range(NTC):
                t0 = tci * TCH
                b = tci // (NTC // B)
                lc = tci % (NTC // B)      # chunk index within batch
                t0b = lc * TCH             # within-batch token offset
                x_t = []
                for xh in range(4):
                    xt_h = xs.tile([P, KC * TCH // 4], bf, tag="xt",
                                   name=f"xt{tci}_{xh}")
                    nc.scalar.dma_start(
                        out=xt_h[:],
                        in_=xRe[:, (tci * KC + xh * KC // 4) * TCH:
                                (tci * KC + (xh + 1) * KC // 4) * TCH])
                    x_t.append(xt_h)

                vp = [ps_v.tile([P, 2 * DQ], f32, tag="vps",
                                name=f"vp{tci}_{u}") for u in range(2)]
                for m in range(HPC):
                    qp = ps_qk.tile([P, TCH], f32, tag="qkps",
                                    name=f"qp{tci}_{m}")
                    kp = ps_qk.tile([P, TCH], f32, tag="qkps",
                                    name=f"kp{tci}_{m}")
                    vfirst = {}
                    for kk in range(KC):
                        xth = x_t[kk // (KC // 4)]
                        kkl = kk % (KC // 4)
                        xt = xth[:, kkl * TCH:(kkl + 1) * TCH]
                        st = (kk == 0)
                        sp = (kk == KC - 1)
                        nc.tensor.matmul(
                            qp[:],
                            wq_sb[:, kk * DQ + m * HD:kk * DQ + (m + 1) * HD],
                            xt, start=st, stop=sp)
                        nc.tensor.matmul(
                            kp[:],
                            wk_sb[:, kk * DQ + m * HD:kk * DQ + (m + 1) * HD],
                            xt, start=st, stop=sp)
                        if m == 0:
                            for tb in range(TCH // P):
                                mm = nc.tensor.matmul(
                                    vp[tb // 2][:, (tb % 2) * DQ:
                                                (tb % 2 + 1) * DQ],
                                    xth[:, kkl * TCH + tb * P:
                                        kkl * TCH + (tb + 1) * P],
                                    wv_sb[:, kk * DQ:(kk + 1) * DQ],
                                    start=(st and tb % 2 == 0), stop=sp,
                                    skip_group_check=(tb % 2 == 1))
                                if kk == 0:
                                    vfirst[tb] = mm
                    if m == 0:
                        for u in range(2):
                            add_dep_helper(vfirst[u * 2 + 1].ins,
                                           vfirst[u * 2].ins, sync=False,
                                           reason="bank-clear 2nd V group")
                    rope(qp, q_sb[:, m * NT + t0:m * NT + t0 + TCH], t0b)
                    rope(kp, k_sb[:, m * NT + t0:m * NT + t0 + TCH], t0b)
                # V psum -> vaug (token-major, per head)
                for tb in range(TCH // P):
                    i = (t0b + tb * P) // P
                    for h in range(HPC):
                        c0 = _vaug_col(b, i, h)
                        nc.vector.tensor_copy(
                            vaug_sb[:, c0:c0 + HD],
                            vp[tb // 2][:, (tb % 2) * DQ + h * HD:
                                        (tb % 2) * DQ + (h + 1) * HD])

                if causal:
                    attn_chunk(b, lc)
                    if lc % 2 == 1:
                        flush_half(b, lc // 2)
                elif lc == (NTC // B) - 1:
                    # non-causal needs the batch's full K/V first
                    for qc in range(4):
                        attn_chunk(b, qc)
                    flush_half(b, 0)
                    flush_half(b, 1)

            outproj(0, 0)
            outproj(0, 1)
            outproj(1, 0)
            outproj(1, 1)
            if dbg:
                nc.sync.dma_start(out=attnD[:, :], in_=attn_sb[:])
                nc.gpsimd.dma_start(out=agD[:, :], in_=ag[0][1][:, :])
                nc.sync.dma_start(out=qD[:, :], in_=q_sb[:])
                nc.sync.dma_start(out=kD[:, :], in_=k_sb[:])
                nc.sync.dma_start(out=vD[:, :], in_=vaug_sb[:])

    nc.compile()
    return nc


def _host_prep(inputs):
    x = np.ascontiguousarray(np.asarray(inputs["x"], np.float32).reshape(NT, D))
    wq = np.asarray(inputs["wq"], np.float32)
    wk = np.asarray(inputs["wk"], np.float32)
    wv = np.asarray(inputs["wv"], np.float32)
    wo = np.asarray(inputs["wo"], np.float32)
    cos = np.asarray(inputs["freqs_cos"], np.float32)
    sin = np.asarray(inputs["freqs_sin"], np.float32)
    mask = np.asarray(inputs["mask"], np.float32).reshape(S, S)

    tril = np.tril(np.ones((S, S), bool))
    if not mask.any():
        mode = "zeros"
    elif (mask[tril] == 0).all() and (mask[~tril] <= -1e8).all():
        mode = "causal"
    else:
        mode = "general"

    # x packed tci-major: xRe[p, (tci*KC+kk)*TCH + t] = x[tci*TCH+t, kk*P+p]
    xRe = np.ascontiguousarray(
        x.reshape(NTC, TCH, KC, P).transpose(3, 0, 2, 1)
        .reshape(P, NTC * KC * TCH).astype(BF))

    C = np.empty((P, S), np.float32)
    Sn = np.empty((P, S), np.float32)
    C[0::2] = cos.T
    C[1::2] = cos.T
    Sn[0::2] = -sin.T
    Sn[1::2] = sin.T
    cro = np.ascontiguousarray(C.astype(BF))
    sro = np.ascontiguousarray(Sn.astype(BF))
    cst = np.zeros((P, 3 * P), np.float32)
    pr = np.zeros((P, P), np.float32)
    idx = np.arange(0, P, 2)
    pr[idx, idx + 1] = 1.0
    pr[idx + 1, idx] = 1.0
    cst[:, 0:P] = pr
    cst[:, P:2 * P] = np.triu(np.ones((P, P), np.float32))
    cst[:, 2 * P:3 * P] = np.eye(P)
    cst = np.ascontiguousarray(cst.astype(BF))

    def wpack(w, r):
        wT = np.ascontiguousarray(w[r, :].T)
        return np.ascontiguousarray(
            wT.reshape(KC, P, DQ).transpose(1, 0, 2)
            .reshape(P, KC * DQ).astype(BF))

    in_maps = []
    for c in range(N_CORES):
        r = slice(c * DQ, (c + 1) * DQ)
        m = {
            "xRe": xRe,
            "wqp": wpack(wq, r),
            "wkp": wpack(wk, r),
            "wvp": wpack(wv, r),
            "wop": wpack(wo, r),
            "cro": cro,
            "sro": sro,
            "cst": cst,
        }
        if mode == "general":
            m["mskT"] = np.ascontiguousarray(
                (mask.T * math.sqrt(HD)).astype(BF))
        in_maps.append(m)
    return mode, in_maps


LAST_RESULT = None


def kernel(**inputs):
    global LAST_RESULT
    from concourse.bass_utils import run_bass_kernel_spmd

    mode, in_maps = _host_prep(inputs)
    if mode not in _cache:
        _cache[mode] = _build(mode)
    nc = _cache[mode]

    res = run_bass_kernel_spmd(nc, in_maps, list(range(N_CORES)))
    LAST_RESULT = res

    out_full = np.empty((NT, D), np.float32)
    for c in range(N_CORES):
        out_full[:, c * DQ:(c + 1) * DQ] = res.results[c]["outT"].T
    return out_full.reshape(B, S, D)
